# revision 29
# baseline (speedup 1.0000x reference)
"""Trainium2 Bass kernel for the hypernet-Conv3D module.

Strategy (data-parallel over batch, one sample per NeuronCore):
  - The tiny hypernet MLP (~2 MFLOP vs 58 GFLOP for the conv) runs on the
    host in fp32 numpy; it produces per-sample conv weights [32,16,3,3,3] and
    biases [32], repacked into matmul-ready block-Toeplitz layouts (bf16).
  - The 3D conv runs on device as an implicit GEMM.  Per matmul:
      * contraction partitions p = 24*s + 8*kh + c8 (s: 5 depth slots, kh: 3
        row-shifted plane copies prepared on the host, c8: 8 of the 16 input
        channels -- the two channel halves sit side by side in the free dim);
      * output partitions 32*j + c (j: 3 depth planes via the block-Toeplitz
        kd band inside the lhsT, c: 32 output channels);
      * so only 6 accumulating matmuls (2 channel halves x 3 kw column
        shifts) cover all 432 contraction terms for 3 output planes -- 2.0
        streamed PE columns per output position instead of 9/4 = 2.25.
  - x is pre-padded and pre-shifted on the host to [68, 3, 8, 2, 4356] bf16
    (zero planes bracketing d, 66x66 zero-padded hw planes, 3 row shifts),
    so each window's input DMA is one fully contiguous [120 x 17424B]
    transfer and SBUF needs no zero maintenance at all.
  - PSUM chunks of 7 padded rows (462 cols); the ScalarE PSUM->SBUF
    evacuation (bias fused) compacts 66-wide padded rows to 64-wide valid
    rows, casting to bf16.  Output y is written as bf16 (one contiguous
    0.75 MB DMA per window), upcast to f32 on the host.
  - Scheduling: input DMAs trigger from the SP queue, output DMAs from the
    Activation queue, so a blocked output trigger can never head-of-line
    block input prefetch.  A run of small warm-up matmuls bridges the initial
    input-DMA latency so the PE p-state ramp is hot when real work arrives;
    the first window's input lands in two row-split DMAs so its first chunks
    arrive early.  The last window drains per-chunk to shorten the tail.
"""

import numpy as np
import ml_dtypes

import concourse.bacc as bacc
import concourse.mybir as mybir
from concourse.tile import TileContext
from concourse.bass_utils import run_bass_kernel_spmd

B, CIN, COUT, K = 8, 16, 32, 3
D = H = W = 64
NUM_W = CIN * COUT * K**3  # 13824

PW = W + 2          # 66 padded row width
P2 = PW * PW        # 4356 padded plane
GD = 3              # output d-planes per window
NWIN = 21           # 20 windows x 3 planes + last window x 4 planes = 64
SLOTS = 5           # input depth slots per window (GD + 2)
NPAD = D + 1        # 65 pre-padded depth planes (zero plane at 0 only)
NPART = 120         # 24 * SLOTS contraction partitions
XFREE = 2 * P2      # two channel halves side by side
N_CORES = 8
N_WARMUP = 22       # PE p-state warm-up matmuls (64 cols each)
# first window: DMA padded-row ranges, split so early chunks land early
# (chunk at r0 with nr rows needs padded rows r0 .. r0+nr only).
W0ROWS = [(0, 5), (5, 9), (9, 17), (17, 33), (33, 49), (49, 66)]
# second window: halved so its early chunks can start on time
W1ROWS = [(0, 33), (33, 66)]
# (start_padded_row, n_rows) chunks covering padded rows 1..64; the dense
# 64-col streaming makes 8 rows exactly one 512-col PSUM bank.
ROW_CHUNKS = [(1 + 8 * i, 8) for i in range(8)]
# first window: two small leading chunks so compute starts earlier
ROW_CHUNKS_W0 = [(1, 4), (5, 4)] + [(9 + 8 * i, 8) for i in range(7)]
# last window: fine-grained chunks, tapering at the end so the closing
# evacuation + drain chain is as short as possible
ROW_CHUNKS_LAST = [(1 + 4 * i, 4) for i in range(15)] + [(61, 3), (64, 1)]

f32 = mybir.dt.float32
bf16 = mybir.dt.bfloat16
DT = bf16
NPDT = ml_dtypes.bfloat16


# ---------------------------------------------------------------- host side

def _host_hypernet(inputs):
    f = np.asarray(inputs["features"], np.float32)
    fc0_w = np.asarray(inputs["fc0_w"], np.float32)
    fc0_b = np.asarray(inputs["fc0_b"], np.float32)
    fc1_w = np.asarray(inputs["fc1_w"], np.float32)
    fc1_b = np.asarray(inputs["fc1_b"], np.float32)
    a0 = np.float32(np.asarray(inputs["a0"]).reshape(-1)[0])
    a1 = np.float32(np.asarray(inputs["a1"]).reshape(-1)[0])
    wg_w = np.asarray(inputs["wg_w"], np.float32)
    wg_b = np.asarray(inputs["wg_b"], np.float32)
    h = f @ fc0_w.T + fc0_b
    h = np.where(h >= 0, h, a0 * h)
    h = h @ fc1_w.T + fc1_b
    h = np.where(h >= 0, h, a1 * h)
    params = h @ wg_w.T + wg_b
    w = params[:, :NUM_W].reshape(B, COUT, CIN, K, K, K).astype(np.float32)
    bias = params[:, NUM_W:].astype(np.float32)
    return w, bias


def _build_wmat(w):
    """w: [32,16,3,3,3] -> [120, 6*128] lhsT bank (bf16).

    Column bank b = 3*g + kw (128 wide) holds lhsT_b with
      lhsT_b[24*s + 8*kh + c8, 32*j + c] = w[c, 8*g + c8, s - j, kh, kw]
    for 0 <= s - j <= 2, zero elsewhere (block-Toeplitz kd band).  Normal
    windows use only cols [0:96) of each bank (j = 0..2); the last window
    also uses j = 3, whose band is naturally truncated to kd in {0, 1} --
    correct there because its kd = 2 tap reads the d = 64 zero padding.
    """
    wmat = np.zeros((SLOTS, 3, 8, 2, 3, GD + 1, COUT), np.float32)
    # wt[kh, kw, kd, cin, cout]
    wt = np.transpose(w, (3, 4, 2, 1, 0))
    for s in range(SLOTS):
        for j in range(GD + 1):
            kd = s - j
            if 0 <= kd <= 2:
                blk = wt[:, :, kd]  # [kh, kw, 16, 32]
                blk = blk.reshape(3, 3, 2, 8, COUT)  # [kh, kw, g, c8, cout]
                wmat[s, :, :, :, :, j, :] = np.transpose(blk, (0, 3, 2, 1, 4))
    return np.ascontiguousarray(
        wmat.reshape(NPART, 6 * 128).astype(NPDT))


def _build_xpad(xb):
    """xb: [16, 64, 64, 64] f32 -> [68, 3, 8, 2, 4356] bf16.

    Plane index pd holds depth d = pd - 1; pd = 0 is all-zero.
    Copy kh holds the 66x66 zero-padded hw plane shifted so padded row r
    contains plain padded row r + kh - 1 (zeros shifted in at the edges).
    Channel cin = 8*g + c8 lives at [pd, kh, c8, g, :].
    """
    bp = np.zeros((D, 8, 2, PW, PW), NPDT)  # [d, c8, g, 66, 66]
    # xb is [cin, d, h, w]; want [d, c8, g, h, w] with cin = 8g + c8
    xs = np.transpose(xb.astype(NPDT).reshape(2, 8, D, H, W),
                      (2, 1, 0, 3, 4))  # [d, c8, g, h, w]
    bp[:, :, :, 1:H + 1, 1:W + 1] = xs
    xp = np.zeros((NPAD, 3, 8, 2, PW, PW), NPDT)
    for kh in range(3):
        sh = kh - 1
        rlo, rhi = max(0, -sh), min(PW, PW - sh)
        xp[1:D + 1, kh, :, :, rlo:rhi] = np.transpose(
            bp[:, :, :, rlo + sh:rhi + sh], (0, 1, 2, 3, 4))
    return np.ascontiguousarray(xp.reshape(NPAD, 3, 8, 2, P2))


# -------------------------------------------------------------- device side

def _conv_body(tc, xt_d, wm_d, bias_d, y_d):
    nc = tc.nc
    with (
        tc.tile_pool(name="const", bufs=1) as cpool,
        tc.tile_pool(name="xw", bufs=1) as xpool,
        tc.tile_pool(name="osb", bufs=3) as opool,
        tc.tile_pool(name="ps", bufs=6, space="PSUM") as pspool,
        tc.tile_pool(name="psw", bufs=1, space="PSUM") as pswpool,
    ):
        wsb = cpool.tile([NPART, 6 * 128], DT, name="wsb")
        nc.sync.dma_start(out=wsb, in_=wm_d[:, :])
        # bias via the Pool SWDGE queue: no HWDGE slot taken, so it neither
        # delays the SP input pipeline nor the first PSUM evacuation.
        bsb = cpool.tile([128, 1], f32, name="bsb")
        nc.gpsimd.dma_start(out=bsb, in_=bias_d[:, :])

        xwins = [
            xpool.tile([NPART, XFREE], DT, name=f"xwin{i}", tag=f"xwin{i}")
            for i in range(3)
        ]

        # PE p-state warm-up: small matmuls on the weight bank keep the
        # tensor engine continuously busy through its frequency ramp while
        # the first input window is still in flight.
        psw = pswpool.tile([96, 64], f32, name="psw", tag="psw")
        for _ in range(N_WARMUP):
            nc.tensor.matmul(
                psw[:, :], lhsT=wsb[:, 0:96], rhs=wsb[:, 0:64],
                start=True, stop=True,
            )

        for win in range(NWIN):
            d0 = GD * win
            xw = xwins[win % 3]
            last = win == NWIN - 1
            # The last window emits a 4th plane (depth 63) on PSUM partitions
            # [96:128): its kd = 2 tap would read the d = 64 zero pad, so its
            # truncated Toeplitz band (kd in {0, 1}) is exact.
            nout = 128 if last else 96
            nj = 4 if last else GD
            # depth slots d0-1 .. d0+3 live at xt_d[d0 .. d0+5) (pre-padded d
            # axis is shifted by one, zero plane at 0).
            src = xt_d[d0:d0 + SLOTS].rearrange("d kh c g n -> (d kh c) (g n)")
            if win <= 1:
                # split so early chunks' rows land early
                dstv = xw[:, :].rearrange("p (g n) -> p g n", g=2)
                srcv = src.rearrange("p (g n) -> p g n", g=2)
                for rlo, rhi in (W0ROWS if win == 0 else W1ROWS):
                    nc.sync.dma_start(
                        out=dstv[:, :, rlo * PW:rhi * PW],
                        in_=srcv[:, :, rlo * PW:rhi * PW])
            else:
                nc.sync.dma_start(out=xw[:, :], in_=src)

            osb = opool.tile([128, D * W], DT, name="osb", tag="osb")
            # [partition, padded row, padded col] views of the two halves
            xv = [
                xw[:, g * P2:(g + 1) * P2].rearrange("p (r z) -> p r z", z=PW)
                for g in range(2)
            ]
            if win == 0:
                chunks = ROW_CHUNKS_W0
            elif last:
                chunks = ROW_CHUNKS_LAST
            else:
                chunks = ROW_CHUNKS
            for ci, (r0, nr) in enumerate(chunks):
                n = nr * W
                ps = pspool.tile([128, 512], f32, name="ps", tag="ps")
                for i in range(6):
                    g, kw = divmod(i, 3)
                    # dense stream: only the 64 valid columns of each padded
                    # row; the kw tap is a column offset inside the 66-row
                    nc.tensor.matmul(
                        ps[0:nout, :n],
                        lhsT=wsb[:, 128 * i:128 * i + nout],
                        rhs=xv[g][:, r0:r0 + nr, kw:kw + W],
                        start=(i == 0),
                        stop=(i == 5),
                    )
                # PSUM -> SBUF: fused bias add + bf16 cast (all contiguous)
                nc.scalar.activation(
                    out=osb[0:nout, (r0 - 1) * W:(r0 - 1 + nr) * W],
                    in_=ps[0:nout, :n],
                    func=mybir.ActivationFunctionType.Identity,
                    bias=bsb[0:nout, 0:1],
                    scale=1.0,
                )
                if last and (ci % 2 == 1 or ci >= 14):
                    # drain evacuated chunks (pairs, then the final three
                    # singly) from the (idle by now) SP queue, shortening
                    # the tail.
                    pair = ci % 2 == 1 and ci < 14
                    rd = r0 - 1 - (4 if pair else 0)
                    rn = nr + (4 if pair else 0)
                    dst = y_d[:, d0:d0 + 4, rd:rd + rn]
                    dst = dst.rearrange("c j h w -> j c (h w)")
                    nc.sync.dma_start(
                        out=dst, in_=osb[:, rd * W:(rd + rn) * W])

            if not last:
                # one contiguous DMA from the Activation queue so it can
                # never block input prefetch on the SP queue:
                # partition 32j+c -> y[c, d0+j, :, :]
                dst = y_d[:, d0:d0 + GD].rearrange("c j h w -> j c (h w)")
                nc.scalar.dma_start(out=dst, in_=osb[0:96, :])


_NC_CACHE = {}


def _get_nc():
    if "nc" not in _NC_CACHE:
        nc = bacc.Bacc("TRN2", target_bir_lowering=False, debug=False)
        xt_d = nc.dram_tensor(
            "xt", [NPAD, 3, 8, 2, P2], DT, kind="ExternalInput")
        wm_d = nc.dram_tensor("wmat", [NPART, 6 * 128], DT, kind="ExternalInput")
        bias_d = nc.dram_tensor("bias", [128, 1], f32, kind="ExternalInput")
        y_d = nc.dram_tensor("y", [COUT, D, H, W], DT, kind="ExternalOutput")
        with TileContext(nc) as tc:
            _conv_body(tc, xt_d, wm_d, bias_d, y_d)
        nc.finalize()  # runs Bacc regalloc/DCE passes, then freezes
        _NC_CACHE["nc"] = nc
    return _NC_CACHE["nc"]


def _run(inputs, trace=False):
    w, bias = _host_hypernet(inputs)
    x = np.asarray(inputs["x"], np.float32)
    in_maps = []
    for b in range(B):
        in_maps.append({
            "xt": _build_xpad(x[b]),
            "wmat": _build_wmat(w[b]),
            "bias": np.ascontiguousarray(np.tile(bias[b], 4).reshape(128, 1)),
        })
    nc = _get_nc()
    res = run_bass_kernel_spmd(
        nc, in_maps, core_ids=list(range(N_CORES)), trace=trace,
    )
    y = np.stack([res.results[b]["y"].astype(np.float32) for b in range(B)])
    return y, res


def kernel(**inputs) -> np.ndarray:
    y, _ = _run(inputs, trace=False)
    return y


# revision 31
# speedup vs baseline: 1.0030x; 1.0030x over previous
"""Trainium2 Bass kernel for the hypernet-Conv3D module.

Strategy (data-parallel over batch, one sample per NeuronCore):
  - The tiny hypernet MLP (~2 MFLOP vs 58 GFLOP for the conv) runs on the
    host in fp32 numpy; it produces per-sample conv weights [32,16,3,3,3] and
    biases [32], repacked into matmul-ready block-Toeplitz layouts (bf16).
  - The 3D conv runs on device as an implicit GEMM.  Per matmul:
      * contraction partitions p = 24*s + 8*kh + c8 (s: 5 depth slots, kh: 3
        row-shifted plane copies prepared on the host, c8: 8 of the 16 input
        channels -- the two channel halves sit side by side in the free dim);
      * output partitions 32*j + c (j: 3 depth planes via the block-Toeplitz
        kd band inside the lhsT, c: 32 output channels);
      * so only 6 accumulating matmuls (2 channel halves x 3 kw column
        shifts) cover all 432 contraction terms for 3 output planes -- 2.0
        streamed PE columns per output position instead of 9/4 = 2.25.
  - x is pre-padded and pre-shifted on the host to [68, 3, 8, 2, 4356] bf16
    (zero planes bracketing d, 66x66 zero-padded hw planes, 3 row shifts),
    so each window's input DMA is one fully contiguous [120 x 17424B]
    transfer and SBUF needs no zero maintenance at all.
  - PSUM chunks of 7 padded rows (462 cols); the ScalarE PSUM->SBUF
    evacuation (bias fused) compacts 66-wide padded rows to 64-wide valid
    rows, casting to bf16.  Output y is written as bf16 (one contiguous
    0.75 MB DMA per window), upcast to f32 on the host.
  - Scheduling: input DMAs trigger from the SP queue, output DMAs from the
    Activation queue, so a blocked output trigger can never head-of-line
    block input prefetch.  A run of small warm-up matmuls bridges the initial
    input-DMA latency so the PE p-state ramp is hot when real work arrives;
    the first window's input lands in two row-split DMAs so its first chunks
    arrive early.  The last window drains per-chunk to shorten the tail.
"""

import numpy as np
import ml_dtypes

import concourse.bacc as bacc
import concourse.mybir as mybir
from concourse.tile import TileContext
from concourse.bass_utils import run_bass_kernel_spmd

B, CIN, COUT, K = 8, 16, 32, 3
D = H = W = 64
NUM_W = CIN * COUT * K**3  # 13824

PW = W + 2          # 66 padded row width
P2 = PW * PW        # 4356 padded plane
GD = 3              # output d-planes per window
NWIN = 21           # 20 windows x 3 planes + last window x 4 planes = 64
SLOTS = 5           # input depth slots per window (GD + 2)
NPAD = D + 1        # 65 pre-padded depth planes (zero plane at 0 only)
NPART = 120         # 24 * SLOTS contraction partitions
XFREE = 2 * P2      # two channel halves side by side
N_CORES = 8
N_WARMUP = 22       # PE p-state warm-up matmuls (64 cols each)
# first window: DMA padded-row ranges, split so early chunks land early
# (chunk at r0 with nr rows needs padded rows r0 .. r0+nr only).
W0ROWS = [(0, 5), (5, 9), (9, 17), (17, 33), (33, 49), (49, 66)]
# second window: halved so its early chunks can start on time
W1ROWS = [(0, 33), (33, 66)]
# (start_padded_row, n_rows) chunks covering padded rows 1..64; the dense
# 64-col streaming makes 8 rows exactly one 512-col PSUM bank.
ROW_CHUNKS = [(1 + 8 * i, 8) for i in range(8)]
# first window: two small leading chunks so compute starts earlier
ROW_CHUNKS_W0 = [(1, 4), (5, 4)] + [(9 + 8 * i, 8) for i in range(7)]
# last window: fine-grained chunks so the final drain DMA is tiny
ROW_CHUNKS_LAST = [(1 + 4 * i, 4) for i in range(16)]

f32 = mybir.dt.float32
bf16 = mybir.dt.bfloat16
DT = bf16
NPDT = ml_dtypes.bfloat16


# ---------------------------------------------------------------- host side

def _host_hypernet(inputs):
    f = np.asarray(inputs["features"], np.float32)
    fc0_w = np.asarray(inputs["fc0_w"], np.float32)
    fc0_b = np.asarray(inputs["fc0_b"], np.float32)
    fc1_w = np.asarray(inputs["fc1_w"], np.float32)
    fc1_b = np.asarray(inputs["fc1_b"], np.float32)
    a0 = np.float32(np.asarray(inputs["a0"]).reshape(-1)[0])
    a1 = np.float32(np.asarray(inputs["a1"]).reshape(-1)[0])
    wg_w = np.asarray(inputs["wg_w"], np.float32)
    wg_b = np.asarray(inputs["wg_b"], np.float32)
    h = f @ fc0_w.T + fc0_b
    h = np.where(h >= 0, h, a0 * h)
    h = h @ fc1_w.T + fc1_b
    h = np.where(h >= 0, h, a1 * h)
    params = h @ wg_w.T + wg_b
    w = params[:, :NUM_W].reshape(B, COUT, CIN, K, K, K).astype(np.float32)
    bias = params[:, NUM_W:].astype(np.float32)
    return w, bias


def _build_wmat(w):
    """w: [32,16,3,3,3] -> [120, 6*128] lhsT bank (bf16).

    Column bank b = 3*g + kw (128 wide) holds lhsT_b with
      lhsT_b[24*s + 8*kh + c8, 32*j + c] = w[c, 8*g + c8, s - j, kh, kw]
    for 0 <= s - j <= 2, zero elsewhere (block-Toeplitz kd band).  Normal
    windows use only cols [0:96) of each bank (j = 0..2); the last window
    also uses j = 3, whose band is naturally truncated to kd in {0, 1} --
    correct there because its kd = 2 tap reads the d = 64 zero padding.
    """
    wmat = np.zeros((SLOTS, 3, 8, 2, 3, GD + 1, COUT), np.float32)
    # wt[kh, kw, kd, cin, cout]
    wt = np.transpose(w, (3, 4, 2, 1, 0))
    for s in range(SLOTS):
        for j in range(GD + 1):
            kd = s - j
            if 0 <= kd <= 2:
                blk = wt[:, :, kd]  # [kh, kw, 16, 32]
                blk = blk.reshape(3, 3, 2, 8, COUT)  # [kh, kw, g, c8, cout]
                wmat[s, :, :, :, :, j, :] = np.transpose(blk, (0, 3, 2, 1, 4))
    return np.ascontiguousarray(
        wmat.reshape(NPART, 6 * 128).astype(NPDT))


def _build_xpad(xb):
    """xb: [16, 64, 64, 64] f32 -> [68, 3, 8, 2, 4356] bf16.

    Plane index pd holds depth d = pd - 1; pd = 0 is all-zero.
    Copy kh holds the 66x66 zero-padded hw plane shifted so padded row r
    contains plain padded row r + kh - 1 (zeros shifted in at the edges).
    Channel cin = 8*g + c8 lives at [pd, kh, c8, g, :].
    """
    bp = np.zeros((D, 8, 2, PW, PW), NPDT)  # [d, c8, g, 66, 66]
    # xb is [cin, d, h, w]; want [d, c8, g, h, w] with cin = 8g + c8
    xs = np.transpose(xb.astype(NPDT).reshape(2, 8, D, H, W),
                      (2, 1, 0, 3, 4))  # [d, c8, g, h, w]
    bp[:, :, :, 1:H + 1, 1:W + 1] = xs
    xp = np.zeros((NPAD, 3, 8, 2, PW, PW), NPDT)
    for kh in range(3):
        sh = kh - 1
        rlo, rhi = max(0, -sh), min(PW, PW - sh)
        xp[1:D + 1, kh, :, :, rlo:rhi] = np.transpose(
            bp[:, :, :, rlo + sh:rhi + sh], (0, 1, 2, 3, 4))
    return np.ascontiguousarray(xp.reshape(NPAD, 3, 8, 2, P2))


# -------------------------------------------------------------- device side

def _conv_body(tc, xt_d, wm_d, bias_d, y_d):
    nc = tc.nc
    with (
        tc.tile_pool(name="const", bufs=1) as cpool,
        tc.tile_pool(name="xw", bufs=1) as xpool,
        tc.tile_pool(name="osb", bufs=3) as opool,
        tc.tile_pool(name="ps", bufs=6, space="PSUM") as pspool,
        tc.tile_pool(name="psw", bufs=1, space="PSUM") as pswpool,
    ):
        wsb = cpool.tile([NPART, 6 * 128], DT, name="wsb")
        nc.sync.dma_start(out=wsb, in_=wm_d[:, :])
        # bias via the Pool SWDGE queue: no HWDGE slot taken, so it neither
        # delays the SP input pipeline nor the first PSUM evacuation.
        bsb = cpool.tile([128, 1], f32, name="bsb")
        nc.gpsimd.dma_start(out=bsb, in_=bias_d[:, :])

        xwins = [
            xpool.tile([NPART, XFREE], DT, name=f"xwin{i}", tag=f"xwin{i}")
            for i in range(3)
        ]

        # PE p-state warm-up: small matmuls on the weight bank keep the
        # tensor engine continuously busy through its frequency ramp while
        # the first input window is still in flight.
        psw = pswpool.tile([96, 64], f32, name="psw", tag="psw")
        for _ in range(N_WARMUP):
            nc.tensor.matmul(
                psw[:, :], lhsT=wsb[:, 0:96], rhs=wsb[:, 0:64],
                start=True, stop=True,
            )

        for win in range(NWIN):
            d0 = GD * win
            xw = xwins[win % 3]
            last = win == NWIN - 1
            # The last window emits a 4th plane (depth 63) on PSUM partitions
            # [96:128): its kd = 2 tap would read the d = 64 zero pad, so its
            # truncated Toeplitz band (kd in {0, 1}) is exact.
            nout = 128 if last else 96
            nj = 4 if last else GD
            # depth slots d0-1 .. d0+3 live at xt_d[d0 .. d0+5) (pre-padded d
            # axis is shifted by one, zero plane at 0).
            src = xt_d[d0:d0 + SLOTS].rearrange("d kh c g n -> (d kh c) (g n)")
            if win <= 1:
                # split so early chunks' rows land early
                dstv = xw[:, :].rearrange("p (g n) -> p g n", g=2)
                srcv = src.rearrange("p (g n) -> p g n", g=2)
                for rlo, rhi in (W0ROWS if win == 0 else W1ROWS):
                    nc.sync.dma_start(
                        out=dstv[:, :, rlo * PW:rhi * PW],
                        in_=srcv[:, :, rlo * PW:rhi * PW])
            else:
                nc.sync.dma_start(out=xw[:, :], in_=src)

            osb = opool.tile([128, D * W], DT, name="osb", tag="osb")
            # [partition, padded row, padded col] views of the two halves
            xv = [
                xw[:, g * P2:(g + 1) * P2].rearrange("p (r z) -> p r z", z=PW)
                for g in range(2)
            ]
            if win == 0:
                chunks = ROW_CHUNKS_W0
            elif last:
                chunks = ROW_CHUNKS_LAST
            else:
                chunks = ROW_CHUNKS
            for ci, (r0, nr) in enumerate(chunks):
                n = nr * W
                ps = pspool.tile([128, 512], f32, name="ps", tag="ps")
                for i in range(6):
                    g, kw = divmod(i, 3)
                    # dense stream: only the 64 valid columns of each padded
                    # row; the kw tap is a column offset inside the 66-row
                    nc.tensor.matmul(
                        ps[0:nout, :n],
                        lhsT=wsb[:, 128 * i:128 * i + nout],
                        rhs=xv[g][:, r0:r0 + nr, kw:kw + W],
                        start=(i == 0),
                        stop=(i == 5),
                    )
                # PSUM -> SBUF: fused bias add + bf16 cast (all contiguous)
                nc.scalar.activation(
                    out=osb[0:nout, (r0 - 1) * W:(r0 - 1 + nr) * W],
                    in_=ps[0:nout, :n],
                    func=mybir.ActivationFunctionType.Identity,
                    bias=bsb[0:nout, 0:1],
                    scale=1.0,
                )
                if last and (ci % 2 == 1 or ci >= 14):
                    # drain evacuated chunks (pairs, then the final two
                    # singly) from the (idle by now) SP queue, shortening
                    # the tail.
                    pair = ci % 2 == 1 and ci < 14
                    rd = r0 - 1 - (nr if pair else 0)
                    rn = nr * (2 if pair else 1)
                    dst = y_d[:, d0:d0 + 4, rd:rd + rn]
                    dst = dst.rearrange("c j h w -> j c (h w)")
                    nc.sync.dma_start(
                        out=dst, in_=osb[:, rd * W:(rd + rn) * W])

            if not last:
                # one contiguous DMA from the Activation queue so it can
                # never block input prefetch on the SP queue:
                # partition 32j+c -> y[c, d0+j, :, :]
                dst = y_d[:, d0:d0 + GD].rearrange("c j h w -> j c (h w)")
                nc.scalar.dma_start(out=dst, in_=osb[0:96, :])


_NC_CACHE = {}


def _get_nc():
    if "nc" not in _NC_CACHE:
        nc = bacc.Bacc("TRN2", target_bir_lowering=False, debug=False)
        xt_d = nc.dram_tensor(
            "xt", [NPAD, 3, 8, 2, P2], DT, kind="ExternalInput")
        wm_d = nc.dram_tensor("wmat", [NPART, 6 * 128], DT, kind="ExternalInput")
        bias_d = nc.dram_tensor("bias", [128, 1], f32, kind="ExternalInput")
        y_d = nc.dram_tensor("y", [COUT, D, H, W], DT, kind="ExternalOutput")
        with TileContext(nc) as tc:
            _conv_body(tc, xt_d, wm_d, bias_d, y_d)
        nc.finalize()  # runs Bacc regalloc/DCE passes, then freezes
        _NC_CACHE["nc"] = nc
    return _NC_CACHE["nc"]


def _run(inputs, trace=False):
    w, bias = _host_hypernet(inputs)
    x = np.asarray(inputs["x"], np.float32)
    in_maps = []
    for b in range(B):
        in_maps.append({
            "xt": _build_xpad(x[b]),
            "wmat": _build_wmat(w[b]),
            "bias": np.ascontiguousarray(np.tile(bias[b], 4).reshape(128, 1)),
        })
    nc = _get_nc()
    res = run_bass_kernel_spmd(
        nc, in_maps, core_ids=list(range(N_CORES)), trace=trace,
    )
    y = np.stack([res.results[b]["y"].astype(np.float32) for b in range(B)])
    return y, res


def kernel(**inputs) -> np.ndarray:
    y, _ = _run(inputs, trace=False)
    return y


# revision 33
# speedup vs baseline: 1.0039x; 1.0009x over previous
"""Trainium2 Bass kernel for the hypernet-Conv3D module.

Strategy (data-parallel over batch, one sample per NeuronCore):
  - The tiny hypernet MLP (~2 MFLOP vs 58 GFLOP for the conv) runs on the
    host in fp32 numpy; it produces per-sample conv weights [32,16,3,3,3] and
    biases [32], repacked into matmul-ready block-Toeplitz layouts (bf16).
  - The 3D conv runs on device as an implicit GEMM.  Per matmul:
      * contraction partitions p = 24*s + 8*kh + c8 (s: 5 depth slots, kh: 3
        row-shifted plane copies prepared on the host, c8: 8 of the 16 input
        channels -- the two channel halves sit side by side in the free dim);
      * output partitions 32*j + c (j: 3 depth planes via the block-Toeplitz
        kd band inside the lhsT, c: 32 output channels);
      * so only 6 accumulating matmuls (2 channel halves x 3 kw column
        shifts) cover all 432 contraction terms for 3 output planes -- 2.0
        streamed PE columns per output position instead of 9/4 = 2.25.
  - x is pre-padded and pre-shifted on the host to [68, 3, 8, 2, 4356] bf16
    (zero planes bracketing d, 66x66 zero-padded hw planes, 3 row shifts),
    so each window's input DMA is one fully contiguous [120 x 17424B]
    transfer and SBUF needs no zero maintenance at all.
  - PSUM chunks of 7 padded rows (462 cols); the ScalarE PSUM->SBUF
    evacuation (bias fused) compacts 66-wide padded rows to 64-wide valid
    rows, casting to bf16.  Output y is written as bf16 (one contiguous
    0.75 MB DMA per window), upcast to f32 on the host.
  - Scheduling: input DMAs trigger from the SP queue, output DMAs from the
    Activation queue, so a blocked output trigger can never head-of-line
    block input prefetch.  A run of small warm-up matmuls bridges the initial
    input-DMA latency so the PE p-state ramp is hot when real work arrives;
    the first window's input lands in two row-split DMAs so its first chunks
    arrive early.  The last window drains per-chunk to shorten the tail.
"""

import numpy as np
import ml_dtypes

import concourse.bacc as bacc
import concourse.mybir as mybir
from concourse.tile import TileContext
from concourse.bass_utils import run_bass_kernel_spmd

B, CIN, COUT, K = 8, 16, 32, 3
D = H = W = 64
NUM_W = CIN * COUT * K**3  # 13824

PW = W + 2          # 66 padded row width
P2 = PW * PW        # 4356 padded plane
GD = 3              # output d-planes per window
NWIN = 21           # 20 windows x 3 planes + last window x 4 planes = 64
SLOTS = 5           # input depth slots per window (GD + 2)
NPAD = D + 1        # 65 pre-padded depth planes (zero plane at 0 only)
NPART = 120         # 24 * SLOTS contraction partitions
XFREE = 2 * P2      # two channel halves side by side
N_CORES = 8
N_WARMUP = 22       # PE p-state warm-up matmuls (64 cols each)
# first window: DMA padded-row ranges, split so early chunks land early
# (chunk at r0 with nr rows needs padded rows r0 .. r0+nr only).
W0ROWS = [(0, 5), (5, 9), (9, 15), (15, 21), (21, 27), (27, 33),
          (33, 49), (49, 66)]
# second window: halved so its early chunks can start on time
W1ROWS = [(0, 33), (33, 66)]
# (start_padded_row, n_rows) chunks covering padded rows 1..64; the dense
# 64-col streaming makes 8 rows exactly one 512-col PSUM bank.
ROW_CHUNKS = [(1 + 8 * i, 8) for i in range(8)]
# first window: graduated chunks so compute starts earlier and never
# outruns the split input DMA pieces
ROW_CHUNKS_W0 = ([(1, 4), (5, 4)] + [(9 + 6 * i, 6) for i in range(4)]
                 + [(33 + 8 * i, 8) for i in range(4)])
# last window: fine-grained chunks so the final drain DMA is tiny
ROW_CHUNKS_LAST = [(1 + 4 * i, 4) for i in range(16)]

f32 = mybir.dt.float32
bf16 = mybir.dt.bfloat16
DT = bf16
NPDT = ml_dtypes.bfloat16


# ---------------------------------------------------------------- host side

def _host_hypernet(inputs):
    f = np.asarray(inputs["features"], np.float32)
    fc0_w = np.asarray(inputs["fc0_w"], np.float32)
    fc0_b = np.asarray(inputs["fc0_b"], np.float32)
    fc1_w = np.asarray(inputs["fc1_w"], np.float32)
    fc1_b = np.asarray(inputs["fc1_b"], np.float32)
    a0 = np.float32(np.asarray(inputs["a0"]).reshape(-1)[0])
    a1 = np.float32(np.asarray(inputs["a1"]).reshape(-1)[0])
    wg_w = np.asarray(inputs["wg_w"], np.float32)
    wg_b = np.asarray(inputs["wg_b"], np.float32)
    h = f @ fc0_w.T + fc0_b
    h = np.where(h >= 0, h, a0 * h)
    h = h @ fc1_w.T + fc1_b
    h = np.where(h >= 0, h, a1 * h)
    params = h @ wg_w.T + wg_b
    w = params[:, :NUM_W].reshape(B, COUT, CIN, K, K, K).astype(np.float32)
    bias = params[:, NUM_W:].astype(np.float32)
    return w, bias


def _build_wmat(w):
    """w: [32,16,3,3,3] -> [120, 6*128] lhsT bank (bf16).

    Column bank b = 3*g + kw (128 wide) holds lhsT_b with
      lhsT_b[24*s + 8*kh + c8, 32*j + c] = w[c, 8*g + c8, s - j, kh, kw]
    for 0 <= s - j <= 2, zero elsewhere (block-Toeplitz kd band).  Normal
    windows use only cols [0:96) of each bank (j = 0..2); the last window
    also uses j = 3, whose band is naturally truncated to kd in {0, 1} --
    correct there because its kd = 2 tap reads the d = 64 zero padding.
    """
    wmat = np.zeros((SLOTS, 3, 8, 2, 3, GD + 1, COUT), np.float32)
    # wt[kh, kw, kd, cin, cout]
    wt = np.transpose(w, (3, 4, 2, 1, 0))
    for s in range(SLOTS):
        for j in range(GD + 1):
            kd = s - j
            if 0 <= kd <= 2:
                blk = wt[:, :, kd]  # [kh, kw, 16, 32]
                blk = blk.reshape(3, 3, 2, 8, COUT)  # [kh, kw, g, c8, cout]
                wmat[s, :, :, :, :, j, :] = np.transpose(blk, (0, 3, 2, 1, 4))
    return np.ascontiguousarray(
        wmat.reshape(NPART, 6 * 128).astype(NPDT))


def _build_xpad(xb):
    """xb: [16, 64, 64, 64] f32 -> [68, 3, 8, 2, 4356] bf16.

    Plane index pd holds depth d = pd - 1; pd = 0 is all-zero.
    Copy kh holds the 66x66 zero-padded hw plane shifted so padded row r
    contains plain padded row r + kh - 1 (zeros shifted in at the edges).
    Channel cin = 8*g + c8 lives at [pd, kh, c8, g, :].
    """
    bp = np.zeros((D, 8, 2, PW, PW), NPDT)  # [d, c8, g, 66, 66]
    # xb is [cin, d, h, w]; want [d, c8, g, h, w] with cin = 8g + c8
    xs = np.transpose(xb.astype(NPDT).reshape(2, 8, D, H, W),
                      (2, 1, 0, 3, 4))  # [d, c8, g, h, w]
    bp[:, :, :, 1:H + 1, 1:W + 1] = xs
    xp = np.zeros((NPAD, 3, 8, 2, PW, PW), NPDT)
    for kh in range(3):
        sh = kh - 1
        rlo, rhi = max(0, -sh), min(PW, PW - sh)
        xp[1:D + 1, kh, :, :, rlo:rhi] = np.transpose(
            bp[:, :, :, rlo + sh:rhi + sh], (0, 1, 2, 3, 4))
    return np.ascontiguousarray(xp.reshape(NPAD, 3, 8, 2, P2))


# -------------------------------------------------------------- device side

def _conv_body(tc, xt_d, wm_d, bias_d, y_d):
    nc = tc.nc
    with (
        tc.tile_pool(name="const", bufs=1) as cpool,
        tc.tile_pool(name="xw", bufs=1) as xpool,
        tc.tile_pool(name="osb", bufs=3) as opool,
        tc.tile_pool(name="ps", bufs=6, space="PSUM") as pspool,
        tc.tile_pool(name="psw", bufs=1, space="PSUM") as pswpool,
    ):
        wsb = cpool.tile([NPART, 6 * 128], DT, name="wsb")
        nc.sync.dma_start(out=wsb, in_=wm_d[:, :])
        # bias via the Pool SWDGE queue: no HWDGE slot taken, so it neither
        # delays the SP input pipeline nor the first PSUM evacuation.
        bsb = cpool.tile([128, 1], f32, name="bsb")
        nc.gpsimd.dma_start(out=bsb, in_=bias_d[:, :])

        xwins = [
            xpool.tile([NPART, XFREE], DT, name=f"xwin{i}", tag=f"xwin{i}")
            for i in range(3)
        ]

        # PE p-state warm-up: small matmuls on the weight bank keep the
        # tensor engine continuously busy through its frequency ramp while
        # the first input window is still in flight.
        psw = pswpool.tile([96, 64], f32, name="psw", tag="psw")
        for _ in range(N_WARMUP):
            nc.tensor.matmul(
                psw[:, :], lhsT=wsb[:, 0:96], rhs=wsb[:, 0:64],
                start=True, stop=True,
            )

        for win in range(NWIN):
            d0 = GD * win
            xw = xwins[win % 3]
            last = win == NWIN - 1
            # The last window emits a 4th plane (depth 63) on PSUM partitions
            # [96:128): its kd = 2 tap would read the d = 64 zero pad, so its
            # truncated Toeplitz band (kd in {0, 1}) is exact.
            nout = 128 if last else 96
            nj = 4 if last else GD
            # depth slots d0-1 .. d0+3 live at xt_d[d0 .. d0+5) (pre-padded d
            # axis is shifted by one, zero plane at 0).
            src = xt_d[d0:d0 + SLOTS].rearrange("d kh c g n -> (d kh c) (g n)")
            if win <= 1:
                # split so early chunks' rows land early
                dstv = xw[:, :].rearrange("p (g n) -> p g n", g=2)
                srcv = src.rearrange("p (g n) -> p g n", g=2)
                for rlo, rhi in (W0ROWS if win == 0 else W1ROWS):
                    nc.sync.dma_start(
                        out=dstv[:, :, rlo * PW:rhi * PW],
                        in_=srcv[:, :, rlo * PW:rhi * PW])
            else:
                nc.sync.dma_start(out=xw[:, :], in_=src)

            osb = opool.tile([128, D * W], DT, name="osb", tag="osb")
            # [partition, padded row, padded col] views of the two halves
            xv = [
                xw[:, g * P2:(g + 1) * P2].rearrange("p (r z) -> p r z", z=PW)
                for g in range(2)
            ]
            if win == 0:
                chunks = ROW_CHUNKS_W0
            elif last:
                chunks = ROW_CHUNKS_LAST
            else:
                chunks = ROW_CHUNKS
            for ci, (r0, nr) in enumerate(chunks):
                n = nr * W
                ps = pspool.tile([128, 512], f32, name="ps", tag="ps")
                for i in range(6):
                    g, kw = divmod(i, 3)
                    # dense stream: only the 64 valid columns of each padded
                    # row; the kw tap is a column offset inside the 66-row
                    nc.tensor.matmul(
                        ps[0:nout, :n],
                        lhsT=wsb[:, 128 * i:128 * i + nout],
                        rhs=xv[g][:, r0:r0 + nr, kw:kw + W],
                        start=(i == 0),
                        stop=(i == 5),
                    )
                # PSUM -> SBUF: fused bias add + bf16 cast (all contiguous)
                nc.scalar.activation(
                    out=osb[0:nout, (r0 - 1) * W:(r0 - 1 + nr) * W],
                    in_=ps[0:nout, :n],
                    func=mybir.ActivationFunctionType.Identity,
                    bias=bsb[0:nout, 0:1],
                    scale=1.0,
                )
                if last and (ci % 2 == 1 or ci >= 14):
                    # drain evacuated chunks (pairs, then the final two
                    # singly) from the (idle by now) SP queue, shortening
                    # the tail.
                    pair = ci % 2 == 1 and ci < 14
                    rd = r0 - 1 - (nr if pair else 0)
                    rn = nr * (2 if pair else 1)
                    dst = y_d[:, d0:d0 + 4, rd:rd + rn]
                    dst = dst.rearrange("c j h w -> j c (h w)")
                    nc.sync.dma_start(
                        out=dst, in_=osb[:, rd * W:(rd + rn) * W])

            if not last:
                # one contiguous DMA from the Activation queue so it can
                # never block input prefetch on the SP queue:
                # partition 32j+c -> y[c, d0+j, :, :]
                dst = y_d[:, d0:d0 + GD].rearrange("c j h w -> j c (h w)")
                nc.scalar.dma_start(out=dst, in_=osb[0:96, :])


_NC_CACHE = {}


def _get_nc():
    if "nc" not in _NC_CACHE:
        nc = bacc.Bacc("TRN2", target_bir_lowering=False, debug=False)
        xt_d = nc.dram_tensor(
            "xt", [NPAD, 3, 8, 2, P2], DT, kind="ExternalInput")
        wm_d = nc.dram_tensor("wmat", [NPART, 6 * 128], DT, kind="ExternalInput")
        bias_d = nc.dram_tensor("bias", [128, 1], f32, kind="ExternalInput")
        y_d = nc.dram_tensor("y", [COUT, D, H, W], DT, kind="ExternalOutput")
        with TileContext(nc) as tc:
            _conv_body(tc, xt_d, wm_d, bias_d, y_d)
        nc.finalize()  # runs Bacc regalloc/DCE passes, then freezes
        _NC_CACHE["nc"] = nc
    return _NC_CACHE["nc"]


def _run(inputs, trace=False):
    w, bias = _host_hypernet(inputs)
    x = np.asarray(inputs["x"], np.float32)
    in_maps = []
    for b in range(B):
        in_maps.append({
            "xt": _build_xpad(x[b]),
            "wmat": _build_wmat(w[b]),
            "bias": np.ascontiguousarray(np.tile(bias[b], 4).reshape(128, 1)),
        })
    nc = _get_nc()
    res = run_bass_kernel_spmd(
        nc, in_maps, core_ids=list(range(N_CORES)), trace=trace,
    )
    y = np.stack([res.results[b]["y"].astype(np.float32) for b in range(B)])
    return y, res


def kernel(**inputs) -> np.ndarray:
    y, _ = _run(inputs, trace=False)
    return y
